# revision 7
# baseline (speedup 1.0000x reference)
"""Cross-attention layer (B=2, Lq=Lk=2048, H=1024, NH=16) on 8 TRN2 NeuronCores.

Sharding: core c handles batch b = c//4 and the 4 heads g*4..g*4+3 where
g = c%4 (data parallel on B x tensor parallel on heads).

Per-core device program (SPMD, identical program, different data):
  - Q/K projections computed in transposed layout QT/KT [head_dim, L]
    directly from host-pretransposed activations decT/encT [H, L]
    (contraction over H lives on the partition axis; fp32r matmuls).
  - V projection in native [k, head_dim] layout, stored f16 with an
    appended ones-column (augmented-V trick: P@V_aug also yields the
    softmax denominator as row 64 of the PSUM accumulator).
  - Scores are computed transposed, S^T = K @ Q^T [k, q], so the softmax
    exp (with mask bias per k-partition and 1/8 scale fused into the ACT
    instruction) needs no reductions at all.
  - E = exp(S^T) f16 goes three ways: DMA to DRAM (the attention-weights
    output, normalized + transposed on host), rhs of the P@V_aug matmul,
    and its column sums (via the ones row) become the denominators.
  - Context rows are normalized with a reciprocal + PE-broadcast and fed
    into the output projection; per-core partial outputs are summed on
    host (tensor-parallel reduce) together with the bias.
"""

import os
import sys

sys.path.insert(0, "/opt/trn_rl_repo")

_PHASES = os.environ.get("KBUILD_PHASES", "full")

import numpy as np

import concourse.bacc as bacc
import concourse.mybir as mybir
import concourse.tile as tile
from concourse.bass_utils import run_bass_kernel_spmd

B, LQ, LK, H, NH = 2, 2048, 2048, 1024, 16
HD = H // NH          # 64
NCORES = 8
HPC = NH // 4         # 4 heads per core
KC = H // 128         # 8 hidden-dim chunks
NKT = LK // 128       # 16 key tiles

F32 = mybir.dt.float32
F32R = mybir.dt.float32r
F16 = mybir.dt.float16
EXP = mybir.ActivationFunctionType.Exp

_NC = None


def _r(ap):
    return ap.bitcast(F32R)


def _emit(nc, dect, enct, wqt, wkt, wvt, wot, mb, ones_d, et, outp, tc):
    with (
        tc.tile_pool(name="xT", bufs=KC) as xt_pool,
        tc.tile_pool(name="wt", bufs=2) as wt_pool,
        tc.tile_pool(name="pp", bufs=1) as pp,
        tc.tile_pool(name="e", bufs=5) as epool,
        tc.tile_pool(name="r", bufs=1) as rpool,
        tc.tile_pool(name="osb", bufs=2) as opool,
        tc.tile_pool(name="ps", bufs=2, space="PSUM") as ps,
        tc.tile_pool(name="pc", bufs=2, space="PSUM") as pc,
    ):
        # ---- persistent tiles ----
        mb_sb = pp.tile([128, NKT], F32, tag="mb")
        nc.sync.dma_start(out=mb_sb, in_=mb[:, :])
        ones_sb = pp.tile([1, HD], F32R, tag="ones")
        nc.sync.dma_start(out=ones_sb, in_=ones_d[:, :])
        wo_sb = pp.tile([HD, HPC, H], F16, tag="wo")
        nc.sync.dma_start(out=wo_sb, in_=wot[:, :, :])

        QT = [pp.tile([128, LQ], F32R, tag=f"qt{j}", name=f"qt{j}") for j in range(2)]
        KT = [pp.tile([128, LK], F32R, tag=f"kt{j}", name=f"kt{j}") for j in range(2)]
        CT = [pp.tile([HD, LQ], F16, tag=f"ct{h}", name=f"ct{h}") for h in range(HPC)]
        v_all = pp.tile([128, NKT, HPC, HD + 1], F16, tag="vall")
        nc.vector.memset(v_all[:, :, :, HD], 1.0)

        # ---- load decT, Q projection ----
        wq_sb = wt_pool.tile([128, KC, 256], F32R, tag="wt")
        nc.sync.dma_start(out=wq_sb, in_=wqt[:, :].rearrange("(c p) m -> p c m", p=128))
        dect_sb = []
        for ci in range(KC):
            t = xt_pool.tile([128, LQ], F32R, tag="xT", name="xt")
            nc.sync.dma_start(out=t, in_=dect[ci * 128:(ci + 1) * 128, :])
            dect_sb.append(t)
        for j in range(2):
            for qb in range(4):
                acc = ps.tile([128, 512], F32, tag="ps", name="acc")
                for ci in range(KC):
                    nc.tensor.matmul(
                        acc,
                        lhsT=wq_sb[:, ci, j * 128:(j + 1) * 128],
                        rhs=dect_sb[ci][:, qb * 512:(qb + 1) * 512],
                        start=(ci == 0),
                        stop=(ci == KC - 1),
                    )
                nc.scalar.copy(out=QT[j][:, qb * 512:(qb + 1) * 512], in_=acc)

        # ---- load encT ----
        enct_sb = []
        for ci in range(KC):
            t = xt_pool.tile([128, LK], F32R, tag="xT", name="xt")
            nc.sync.dma_start(out=t, in_=enct[ci * 128:(ci + 1) * 128, :])
            enct_sb.append(t)

        # ---- V projection (native [k, hd] layout + ones column) ----
        wv_sb = wt_pool.tile([128, KC, 256], F32R, tag="wt")
        nc.sync.dma_start(out=wv_sb, in_=wvt[:, :].rearrange("(c p) m -> p c m", p=128))
        for kt_i in range(NKT):
            acc = ps.tile([128, 256], F32, tag="ps", name="accv")
            for ci in range(KC):
                nc.tensor.matmul(
                    acc,
                    lhsT=enct_sb[ci][:, kt_i * 128:(kt_i + 1) * 128],
                    rhs=wv_sb[:, ci, :],
                    start=(ci == 0),
                    stop=(ci == KC - 1),
                )
            nc.scalar.copy(
                out=v_all[:, kt_i, :, 0:HD],
                in_=acc.rearrange("p (h d) -> p h d", h=HPC),
            )

        # ---- K projection ----
        wk_sb = wt_pool.tile([128, KC, 256], F32R, tag="wt")
        nc.sync.dma_start(out=wk_sb, in_=wkt[:, :].rearrange("(c p) m -> p c m", p=128))
        for j in range(2):
            for kb in range(4):
                acc = ps.tile([128, 512], F32, tag="ps", name="acc")
                for ci in range(KC):
                    nc.tensor.matmul(
                        acc,
                        lhsT=wk_sb[:, ci, j * 128:(j + 1) * 128],
                        rhs=enct_sb[ci][:, kb * 512:(kb + 1) * 512],
                        start=(ci == 0),
                        stop=(ci == KC - 1),
                    )
                nc.scalar.copy(out=KT[j][:, kb * 512:(kb + 1) * 512], in_=acc)

        # ---- attention: per q-half, per head pair ----
        for qh in range(2 if _PHASES != "proj" else 0):
            q0 = qh * 1024
            for j in range(2):
                cps = [pc.tile([HD + 1, 1024], F32, tag="pc", name=f"cps{_s}") for _s in range(2)]
                for ci in range(NKT):
                    k0 = ci * 128
                    ss = [
                        ps.tile([128, 1024], F32, tag="ps", name="s")
                        for _s in range(2)
                    ]
                    # qb-major emission: the sub=0 / sub=1 matmuls hit
                    # disjoint PE row groups and run concurrently.
                    for qb in range(2):
                        for sub in range(2):
                            p0 = sub * 64
                            nc.tensor.matmul(
                                ss[sub][:, qb * 512:(qb + 1) * 512],
                                lhsT=KT[j][p0:p0 + 64, k0:k0 + 128],
                                rhs=QT[j][p0:p0 + 64, q0 + qb * 512:q0 + (qb + 1) * 512],
                                start=True,
                                stop=True,
                            )
                    es = []
                    for sub in range(2):
                        e = epool.tile([128, 1024], F16, tag="e", name="e")
                        nc.scalar.activation(
                            e, ss[sub], EXP, bias=mb_sb[:, ci:ci + 1], scale=0.125
                        )
                        es.append(e)
                    for sub in range(2):
                        h = 2 * j + sub
                        for qb in range(2):
                            nc.tensor.matmul(
                                cps[sub][:, qb * 512:(qb + 1) * 512],
                                lhsT=v_all[:, ci, h, :],
                                rhs=es[sub][:, qb * 512:(qb + 1) * 512],
                                start=(ci == 0),
                                stop=(ci == NKT - 1),
                            )
                        nc.sync.dma_start(
                            out=et[h, k0:k0 + 128, q0:q0 + 1024], in_=es[sub]
                        )
                # normalize context rows: CT_h = C_unnorm / denom
                for sub in range(2 if _PHASES not in ("attn",) else 0):
                    h = 2 * j + sub
                    dn = rpool.tile([1, 1024], F32R, tag="dn", name="dn")
                    nc.scalar.copy(out=dn, in_=cps[sub][HD:HD + 1, :])
                    rb = ps.tile([HD, 1024], F32, tag="ps", name="rb")
                    for qb in range(2):
                        nc.tensor.matmul(
                            rb[:, qb * 512:(qb + 1) * 512],
                            lhsT=ones_sb,
                            rhs=dn[:, qb * 512:(qb + 1) * 512],
                            start=True,
                            stop=True,
                        )
                    rec = rpool.tile([HD, 1024], F32, tag="rec", name="rec")
                    nc.vector.reciprocal_approx_fast(out=rec, in_=rb)
                    nc.vector.tensor_mul(
                        CT[h][:, q0:q0 + 1024], cps[sub][0:HD, :], rec
                    )

            # ---- output projection for this q-half (overlaps next half) ----
            if _PHASES in ("full", "noct"):
                for qt in range(qh * 8, (qh + 1) * 8):
                    for ob in range(2):
                        acc = ps.tile([128, 512], F32, tag="ps", name="acc")
                        for h in range(HPC):
                            nc.tensor.matmul(
                                acc,
                                lhsT=CT[h][:, qt * 128:(qt + 1) * 128],
                                rhs=wo_sb[:, h, ob * 512:(ob + 1) * 512],
                                start=(h == 0),
                                stop=(h == HPC - 1),
                            )
                        o = opool.tile([128, 512], F32, tag="osb", name="o")
                        nc.scalar.copy(out=o, in_=acc)
                        nc.sync.dma_start(
                            out=outp[qt * 128:(qt + 1) * 128, ob * 512:(ob + 1) * 512],
                            in_=o,
                        )




def _build():
    nc = bacc.Bacc(
        "TRN2",
        target_bir_lowering=False,
        debug=False,
        enable_asserts=True,
        num_devices=NCORES,
    )
    dect = nc.declare_dram_parameter("dect", [H, LQ], F32R, isOutput=False)
    enct = nc.declare_dram_parameter("enct", [H, LK], F32R, isOutput=False)
    wqt = nc.declare_dram_parameter("wqt", [H, 256], F32R, isOutput=False)
    wkt = nc.declare_dram_parameter("wkt", [H, 256], F32R, isOutput=False)
    wvt = nc.declare_dram_parameter("wvt", [H, 256], F32R, isOutput=False)
    wot = nc.declare_dram_parameter("wot", [HD, HPC, H], F16, isOutput=False)
    mb = nc.declare_dram_parameter("maskb", [128, NKT], F32, isOutput=False)
    ones_d = nc.declare_dram_parameter("ones", [1, HD], F32R, isOutput=False)
    et = nc.declare_dram_parameter("et", [HPC, LK, LQ], F16, isOutput=True)
    outp = nc.declare_dram_parameter("outp", [LQ, H], F32, isOutput=True)

    with tile.TileContext(nc) as tc:
        _emit(nc, dect, enct, wqt, wkt, wvt, wot, mb, ones_d, et, outp, tc)
    nc.compile()
    return nc


def _get_nc():
    global _NC
    if _NC is None:
        _NC = _build()
    return _NC


def _in_maps(dec, enc, mask, Wq, Wk, Wv, Wo):
    decT = [np.ascontiguousarray(dec[b].T) for b in range(B)]
    encT = [np.ascontiguousarray(enc[b].T) for b in range(B)]
    maskb = []
    for b in range(B):
        bias = np.where(mask[b] != 0, 0.0, -1e30).astype(np.float32)
        maskb.append(np.ascontiguousarray(bias.reshape(NKT, 128).T))
    maps = []
    for c in range(NCORES):
        b, g = c // 4, c % 4
        sl = slice(g * 256, (g + 1) * 256)
        wot_c = np.ascontiguousarray(
            Wo[:, sl].T.reshape(HPC, HD, H).transpose(1, 0, 2).astype(np.float16)
        )
        maps.append(
            {
                "dect": decT[b],
                "enct": encT[b],
                "wqt": np.ascontiguousarray(Wq[sl, :].T),
                "wkt": np.ascontiguousarray(Wk[sl, :].T),
                "wvt": np.ascontiguousarray(Wv[sl, :].T),
                "wot": wot_c,
                "maskb": maskb[b],
                "ones": np.ones((1, HD), np.float32),
            }
        )
    return maps


def _assemble(results, bo):
    out = np.empty((B, LQ, H), np.float32)
    weights = np.empty((B, NH, LQ, LK), np.float32)
    for b in range(B):
        acc = results[b * 4]["outp"].astype(np.float32, copy=True)
        for g in range(1, 4):
            acc += results[b * 4 + g]["outp"]
        out[b] = acc + bo[None, :]
    for c in range(NCORES):
        b, g = c // 4, c % 4
        E16 = results[c]["et"]  # [HPC, k, q] f16
        for hl in range(HPC):
            Ef = E16[hl].astype(np.float32)
            denom = Ef.sum(axis=0)
            weights[b, g * 4 + hl] = (Ef / denom[None, :]).T
    return out, weights


def _run(inputs, trace=False):
    dec = np.asarray(inputs["decoder_hidden"], np.float32)
    enc = np.asarray(inputs["encoder_hidden"], np.float32)
    mask = np.asarray(inputs["encoder_attention_mask"])
    Wq = np.asarray(inputs["Wq"], np.float32)
    Wk = np.asarray(inputs["Wk"], np.float32)
    Wv = np.asarray(inputs["Wv"], np.float32)
    Wo = np.asarray(inputs["Wo"], np.float32)
    bo = np.asarray(inputs["bo"], np.float32)

    nc = _get_nc()
    maps = _in_maps(dec, enc, mask, Wq, Wk, Wv, Wo)
    res = run_bass_kernel_spmd(nc, maps, list(range(NCORES)), trace=trace)
    out, weights = _assemble(res.results, bo)
    return out, weights, res


def kernel(
    decoder_hidden,
    encoder_hidden,
    encoder_attention_mask,
    Wq,
    Wk,
    Wv,
    Wo,
    bo,
):
    out, weights, _ = _run(
        {
            "decoder_hidden": decoder_hidden,
            "encoder_hidden": encoder_hidden,
            "encoder_attention_mask": encoder_attention_mask,
            "Wq": Wq,
            "Wk": Wk,
            "Wv": Wv,
            "Wo": Wo,
            "bo": bo,
        }
    )
    return out, weights


# revision 8
# speedup vs baseline: 1.1937x; 1.1937x over previous
"""Cross-attention layer (B=2, Lq=Lk=2048, H=1024, NH=16) on 8 TRN2 NeuronCores.

Sharding: core c handles batch b = c//4 and the 4 heads g*4..g*4+3 where
g = c%4 (data parallel on B x tensor parallel on heads).

Per-core device program (SPMD, identical program, different data):
  - Q/K projections computed in transposed layout QT/KT [head_dim, L]
    directly from host-pretransposed activations decT/encT [H, L]
    (contraction over H lives on the partition axis; fp32r matmuls).
  - V projection in native [k, head_dim] layout, stored f16 with an
    appended ones-column (augmented-V trick: P@V_aug also yields the
    softmax denominator as row 64 of the PSUM accumulator).
  - Scores are computed transposed, S^T = K @ Q^T [k, q], so the softmax
    exp (with mask bias per k-partition and 1/8 scale fused into the ACT
    instruction) needs no reductions at all. The two heads of a pair use
    disjoint PE row groups (K=64) and are emitted adjacently so they can
    run concurrently on the PE's 32x32 subarrays.
  - The P@V matmul runs one k-chunk behind the scores (software
    pipeline) so the scheduler never splits a score pair.
  - E = exp(S^T) f16 goes three ways: DMA to DRAM (the attention-weights
    output, normalized + transposed on host), rhs of the P@V_aug matmul,
    and its column sums (via the ones row) become the denominators.
  - Context rows are normalized with a reciprocal + PE-broadcast and fed
    into the output projection; per-core partial outputs are summed on
    host (tensor-parallel reduce) together with the bias.
"""

import os
import sys

sys.path.insert(0, "/opt/trn_rl_repo")

import numpy as np

import concourse.bacc as bacc
import concourse.mybir as mybir
import concourse.tile as tile
from concourse.bass_utils import run_bass_kernel_spmd

_PHASES = os.environ.get("KBUILD_PHASES", "full")

B, LQ, LK, H, NH = 2, 2048, 2048, 1024, 16
HD = H // NH          # 64
NCORES = 8
HPC = NH // 4         # 4 heads per core
KC = H // 128         # 8 hidden-dim chunks
NKT = LK // 128       # 16 key tiles

F32 = mybir.dt.float32
F32R = mybir.dt.float32r
F16 = mybir.dt.float16
EXP = mybir.ActivationFunctionType.Exp

_NC = None


def _emit(nc, dect, enct, wqt, wkt, wvt, wot, mb, ones_d, et, outp, tc):
    with (
        tc.tile_pool(name="xT", bufs=KC) as xt_pool,
        tc.tile_pool(name="wt", bufs=2) as wt_pool,
        tc.tile_pool(name="pp", bufs=1) as pp,
        tc.tile_pool(name="e", bufs=5) as epool,
        tc.tile_pool(name="r", bufs=1) as rpool,
        tc.tile_pool(name="osb", bufs=2) as opool,
        tc.tile_pool(name="ps", bufs=2, space="PSUM") as ps,
        tc.tile_pool(name="pc", bufs=2, space="PSUM") as pc,
    ):
        # ---- persistent tiles ----
        mb_sb = pp.tile([128, NKT], F32, tag="mb")
        nc.sync.dma_start(out=mb_sb, in_=mb[:, :])
        ones_sb = pp.tile([1, HD], F32R, tag="ones")
        nc.sync.dma_start(out=ones_sb, in_=ones_d[:, :])
        wo_sb = pp.tile([HD, HPC, H], F16, tag="wo")
        nc.sync.dma_start(out=wo_sb, in_=wot[:, :, :])

        QT = [pp.tile([128, LQ], F32R, tag=f"qt{j}", name=f"qt{j}") for j in range(2)]
        KT = [pp.tile([128, LK], F32R, tag=f"kt{j}", name=f"kt{j}") for j in range(2)]
        CT = [pp.tile([HD, LQ], F16, tag=f"ct{h}", name=f"ct{h}") for h in range(HPC)]
        v_all = pp.tile([128, NKT, HPC, HD + 1], F16, tag="vall")
        nc.vector.memset(v_all[:, :, :, HD], 1.0)

        # ---- load decT, Q projection ----
        wq_sb = wt_pool.tile([128, KC, 256], F32R, tag="wt")
        nc.sync.dma_start(out=wq_sb, in_=wqt[:, :].rearrange("(c p) m -> p c m", p=128))
        dect_sb = []
        for ci in range(KC):
            t = xt_pool.tile([128, LQ], F32R, tag="xT", name="xt")
            nc.sync.dma_start(out=t, in_=dect[ci * 128:(ci + 1) * 128, :])
            dect_sb.append(t)
        for j in range(2):
            for qb in range(4):
                acc = ps.tile([128, 512], F32, tag="ps", name="acc")
                for ci in range(KC):
                    nc.tensor.matmul(
                        acc,
                        lhsT=wq_sb[:, ci, j * 128:(j + 1) * 128],
                        rhs=dect_sb[ci][:, qb * 512:(qb + 1) * 512],
                        start=(ci == 0),
                        stop=(ci == KC - 1),
                    )
                nc.scalar.copy(out=QT[j][:, qb * 512:(qb + 1) * 512], in_=acc)

        # ---- load encT, K projection ----
        wk_sb = wt_pool.tile([128, KC, 256], F32R, tag="wt")
        nc.sync.dma_start(out=wk_sb, in_=wkt[:, :].rearrange("(c p) m -> p c m", p=128))
        enct_sb = []
        for ci in range(KC):
            t = xt_pool.tile([128, LK], F32R, tag="xT", name="xt")
            nc.sync.dma_start(out=t, in_=enct[ci * 128:(ci + 1) * 128, :])
            enct_sb.append(t)
        for j in range(2):
            for kb in range(4):
                acc = ps.tile([128, 512], F32, tag="ps", name="acc")
                for ci in range(KC):
                    nc.tensor.matmul(
                        acc,
                        lhsT=wk_sb[:, ci, j * 128:(j + 1) * 128],
                        rhs=enct_sb[ci][:, kb * 512:(kb + 1) * 512],
                        start=(ci == 0),
                        stop=(ci == KC - 1),
                    )
                nc.scalar.copy(out=KT[j][:, kb * 512:(kb + 1) * 512], in_=acc)

        # ---- V projection (native [k, hd] layout + ones column) ----
        wv_sb = wt_pool.tile([128, KC, 256], F32R, tag="wt")
        nc.sync.dma_start(out=wv_sb, in_=wvt[:, :].rearrange("(c p) m -> p c m", p=128))
        for kt_i in range(NKT):
            acc = ps.tile([128, 256], F32, tag="ps", name="accv")
            for ci in range(KC):
                nc.tensor.matmul(
                    acc,
                    lhsT=enct_sb[ci][:, kt_i * 128:(kt_i + 1) * 128],
                    rhs=wv_sb[:, ci, :],
                    start=(ci == 0),
                    stop=(ci == KC - 1),
                )
            nc.scalar.copy(
                out=v_all[:, kt_i, :, 0:HD],
                in_=acc.rearrange("p (h d) -> p h d", h=HPC),
            )

        # ---- attention: per q-half, per head pair; PV one chunk behind ----
        for qh in range(2 if _PHASES != "proj" else 0):
            q0 = qh * 1024
            for j in range(2):
                cps = [
                    pc.tile([HD + 1, 1024], F32, tag="pc", name=f"cps{s_}")
                    for s_ in range(2)
                ]
                prev = None  # es of previous chunk
                for ci in range(NKT + 1):
                    if ci < NKT:
                        k0 = ci * 128
                        ss = [
                            ps.tile([128, 1024], F32, tag="ps", name="s")
                            for s_ in range(2)
                        ]
                        # qb-major emission: the sub=0 / sub=1 matmuls hit
                        # disjoint PE row groups and run concurrently.
                        for qb in range(2):
                            for sub in range(2):
                                p0 = sub * 64
                                nc.tensor.matmul(
                                    ss[sub][:, qb * 512:(qb + 1) * 512],
                                    lhsT=KT[j][p0:p0 + 64, k0:k0 + 128],
                                    rhs=QT[j][p0:p0 + 64, q0 + qb * 512:q0 + (qb + 1) * 512],
                                    start=True,
                                    stop=True,
                                )
                        es = []
                        for sub in range(2):
                            e = epool.tile([128, 1024], F16, tag="e", name="e")
                            nc.scalar.activation(
                                e, ss[sub], EXP, bias=mb_sb[:, ci:ci + 1], scale=0.125
                            )
                            es.append(e)
                    # PV + weights DMA for the previous chunk
                    if prev is not None:
                        pes = prev
                        pci = ci - 1
                        pk0 = pci * 128
                        for sub in range(2):
                            h = 2 * j + sub
                            for qb in range(2):
                                nc.tensor.matmul(
                                    cps[sub][:, qb * 512:(qb + 1) * 512],
                                    lhsT=v_all[:, pci, h, :],
                                    rhs=pes[sub][:, qb * 512:(qb + 1) * 512],
                                    start=(pci == 0),
                                    stop=(pci == NKT - 1),
                                )
                            nc.sync.dma_start(
                                out=et[h, pk0:pk0 + 128, q0:q0 + 1024], in_=pes[sub]
                            )
                    prev = es if ci < NKT else None
                # normalize context rows: CT_h = C_unnorm / denom
                for sub in range(2 if _PHASES not in ("attn",) else 0):
                    h = 2 * j + sub
                    dn = rpool.tile([1, 1024], F32R, tag="dn", name="dn")
                    nc.scalar.copy(out=dn, in_=cps[sub][HD:HD + 1, :])
                    rb = ps.tile([HD, 1024], F32, tag="ps", name="rb")
                    for qb in range(2):
                        nc.tensor.matmul(
                            rb[:, qb * 512:(qb + 1) * 512],
                            lhsT=ones_sb,
                            rhs=dn[:, qb * 512:(qb + 1) * 512],
                            start=True,
                            stop=True,
                        )
                    rec = rpool.tile([HD, 1024], F32, tag="rec", name="rec")
                    nc.vector.reciprocal_approx_fast(out=rec, in_=rb)
                    nc.vector.tensor_mul(
                        CT[h][:, q0:q0 + 1024], cps[sub][0:HD, :], rec
                    )

        # ---- output projection: outp[q, o] = sum_h CT_h.T @ Wo_h ----
        for qt in range(16 if _PHASES in ("full", "noct") else 0):
            for ob in range(2):
                acc = ps.tile([128, 512], F32, tag="ps", name="acc")
                for h in range(HPC):
                    nc.tensor.matmul(
                        acc,
                        lhsT=CT[h][:, qt * 128:(qt + 1) * 128],
                        rhs=wo_sb[:, h, ob * 512:(ob + 1) * 512],
                        start=(h == 0),
                        stop=(h == HPC - 1),
                    )
                o = opool.tile([128, 512], F32, tag="osb", name="o")
                nc.scalar.copy(out=o, in_=acc)
                nc.sync.dma_start(
                    out=outp[qt * 128:(qt + 1) * 128, ob * 512:(ob + 1) * 512], in_=o
                )


def _build():
    nc = bacc.Bacc(
        "TRN2",
        target_bir_lowering=False,
        debug=False,
        enable_asserts=True,
        num_devices=NCORES,
    )
    dect = nc.declare_dram_parameter("dect", [H, LQ], F32R, isOutput=False)
    enct = nc.declare_dram_parameter("enct", [H, LK], F32R, isOutput=False)
    wqt = nc.declare_dram_parameter("wqt", [H, 256], F32R, isOutput=False)
    wkt = nc.declare_dram_parameter("wkt", [H, 256], F32R, isOutput=False)
    wvt = nc.declare_dram_parameter("wvt", [H, 256], F32R, isOutput=False)
    wot = nc.declare_dram_parameter("wot", [HD, HPC, H], F16, isOutput=False)
    mb = nc.declare_dram_parameter("maskb", [128, NKT], F32, isOutput=False)
    ones_d = nc.declare_dram_parameter("ones", [1, HD], F32R, isOutput=False)
    et = nc.declare_dram_parameter("et", [HPC, LK, LQ], F16, isOutput=True)
    outp = nc.declare_dram_parameter("outp", [LQ, H], F32, isOutput=True)

    with tile.TileContext(nc) as tc:
        _emit(nc, dect, enct, wqt, wkt, wvt, wot, mb, ones_d, et, outp, tc)
    nc.compile()
    return nc


def _get_nc():
    global _NC
    if _NC is None:
        _NC = _build()
    return _NC


def _in_maps(dec, enc, mask, Wq, Wk, Wv, Wo):
    decT = [np.ascontiguousarray(dec[b].T) for b in range(B)]
    encT = [np.ascontiguousarray(enc[b].T) for b in range(B)]
    maskb = []
    for b in range(B):
        bias = np.where(mask[b] != 0, 0.0, -1e30).astype(np.float32)
        maskb.append(np.ascontiguousarray(bias.reshape(NKT, 128).T))
    maps = []
    for c in range(NCORES):
        b, g = c // 4, c % 4
        sl = slice(g * 256, (g + 1) * 256)
        wot_c = np.ascontiguousarray(
            Wo[:, sl].T.reshape(HPC, HD, H).transpose(1, 0, 2).astype(np.float16)
        )
        maps.append(
            {
                "dect": decT[b],
                "enct": encT[b],
                "wqt": np.ascontiguousarray(Wq[sl, :].T),
                "wkt": np.ascontiguousarray(Wk[sl, :].T),
                "wvt": np.ascontiguousarray(Wv[sl, :].T),
                "wot": wot_c,
                "maskb": maskb[b],
                "ones": np.ones((1, HD), np.float32),
            }
        )
    return maps


def _assemble(results, bo):
    out = np.empty((B, LQ, H), np.float32)
    weights = np.empty((B, NH, LQ, LK), np.float32)
    for b in range(B):
        acc = results[b * 4]["outp"].astype(np.float32, copy=True)
        for g in range(1, 4):
            acc += results[b * 4 + g]["outp"]
        out[b] = acc + bo[None, :]
    for c in range(NCORES):
        b, g = c // 4, c % 4
        E16 = results[c]["et"]  # [HPC, k, q] f16
        for hl in range(HPC):
            Ef = E16[hl].astype(np.float32)
            denom = Ef.sum(axis=0)
            weights[b, g * 4 + hl] = (Ef / denom[None, :]).T
    return out, weights


def _run(inputs, trace=False):
    dec = np.asarray(inputs["decoder_hidden"], np.float32)
    enc = np.asarray(inputs["encoder_hidden"], np.float32)
    mask = np.asarray(inputs["encoder_attention_mask"])
    Wq = np.asarray(inputs["Wq"], np.float32)
    Wk = np.asarray(inputs["Wk"], np.float32)
    Wv = np.asarray(inputs["Wv"], np.float32)
    Wo = np.asarray(inputs["Wo"], np.float32)
    bo = np.asarray(inputs["bo"], np.float32)

    nc = _get_nc()
    maps = _in_maps(dec, enc, mask, Wq, Wk, Wv, Wo)
    res = run_bass_kernel_spmd(nc, maps, list(range(NCORES)), trace=trace)
    out, weights = _assemble(res.results, bo)
    return out, weights, res


def kernel(
    decoder_hidden,
    encoder_hidden,
    encoder_attention_mask,
    Wq,
    Wk,
    Wv,
    Wo,
    bo,
):
    out, weights, _ = _run(
        {
            "decoder_hidden": decoder_hidden,
            "encoder_hidden": encoder_hidden,
            "encoder_attention_mask": encoder_attention_mask,
            "Wq": Wq,
            "Wk": Wk,
            "Wv": Wv,
            "Wo": Wo,
            "bo": bo,
        }
    )
    return out, weights


# revision 9
# speedup vs baseline: 1.5379x; 1.2883x over previous
"""Cross-attention layer (B=2, Lq=Lk=2048, H=1024, NH=16) on 8 TRN2 NeuronCores.

Sharding: core c handles batch b = c//4 and the 4 heads g*4..g*4+3 where
g = c%4 (data parallel on B x tensor parallel on heads).

Per-core device program (SPMD, identical program, different data):
  - Q/K projections computed in transposed layout QT/KT [head_dim, L]
    directly from host-pretransposed activations decT/encT [H, L]
    (contraction over H lives on the partition axis; fp32r matmuls).
  - V projection in native [k, head_dim] layout, stored f16 with an
    appended ones-column (augmented-V trick: P@V_aug also yields the
    softmax denominator as row 64 of the PSUM accumulator).
  - Scores are computed transposed, S^T = K @ Q^T [k, q], so the softmax
    exp (with mask bias per k-partition and 1/8 scale fused into the ACT
    instruction) needs no reductions at all. The two heads of a pair use
    disjoint PE row groups (K=64) and are emitted adjacently so they can
    run concurrently on the PE's 32x32 subarrays.
  - The P@V matmul runs one k-chunk behind the scores (software
    pipeline) so the scheduler never splits a score pair.
  - E = exp(S^T) f16 goes three ways: DMA to DRAM (the attention-weights
    output, normalized + transposed on host), rhs of the P@V_aug matmul,
    and its column sums (via the ones row) become the denominators.
  - Context rows are normalized with a reciprocal + PE-broadcast and fed
    into the output projection; per-core partial outputs are summed on
    host (tensor-parallel reduce) together with the bias.
"""

import os
import sys

sys.path.insert(0, "/opt/trn_rl_repo")

import numpy as np

import concourse.bacc as bacc
import concourse.mybir as mybir
import concourse.tile as tile
from concourse.bass_utils import run_bass_kernel_spmd

_PHASES = os.environ.get("KBUILD_PHASES", "full")

B, LQ, LK, H, NH = 2, 2048, 2048, 1024, 16
HD = H // NH          # 64
NCORES = 8
HPC = NH // 4         # 4 heads per core
KC = H // 128         # 8 hidden-dim chunks
NKT = LK // 128       # 16 key tiles

F32 = mybir.dt.float32
F32R = mybir.dt.float32r
F16 = mybir.dt.float16
EXP = mybir.ActivationFunctionType.Exp

_NC = None
_ONES_Z = np.zeros((128, HD), np.float32)
_ONES_Z[0, :] = 1.0
_DN_Z = np.zeros((128, 1024), np.float32)


def _emit(nc, dect, enct, wqt, wkt, wvt, wot, mb, ones_d, dnz_d, et, outp, tc):
    with (
        tc.tile_pool(name="xT", bufs=KC) as xt_pool,
        tc.tile_pool(name="wt", bufs=2) as wt_pool,
        tc.tile_pool(name="pp", bufs=1) as pp,
        tc.tile_pool(name="e", bufs=5) as epool,
        tc.tile_pool(name="r", bufs=1) as rpool,
        tc.tile_pool(name="osb", bufs=2) as opool,
        tc.tile_pool(name="ps", bufs=2, space="PSUM") as ps,
        tc.tile_pool(name="pc", bufs=2, space="PSUM") as pc,
    ):
        # ---- persistent tiles ----
        mb_sb = pp.tile([128, NKT], F32, tag="mb")
        nc.sync.dma_start(out=mb_sb, in_=mb[:, :])
        ones_sb = pp.tile([128, HD], F32R, tag="ones")
        nc.sync.dma_start(out=ones_sb, in_=ones_d[:, :])
        dn = pp.tile([128, 1024], F32R, tag="dn")
        nc.sync.dma_start(out=dn, in_=dnz_d[:, :])
        wo_sb = pp.tile([HD, HPC, H], F16, tag="wo")
        nc.sync.dma_start(out=wo_sb, in_=wot[:, :, :])

        QT = [pp.tile([128, LQ], F32R, tag=f"qt{j}", name=f"qt{j}") for j in range(2)]
        # per-head K tiles, zero-padded on the other head's partitions so the
        # score matmuls run with K=128 (no PE tiling-mode switches)
        KT = [pp.tile([128, LK], F32R, tag=f"kt{h}", name=f"kt{h}") for h in range(HPC)]
        CT = [pp.tile([HD, LQ], F16, tag=f"ct{h}", name=f"ct{h}") for h in range(HPC)]
        v_all = pp.tile([128, NKT, HPC, HD + 1], F16, tag="vall")
        nc.vector.memset(v_all[:, :, :, HD], 1.0)

        # ---- load decT, Q projection ----
        wq_sb = wt_pool.tile([128, KC, 256], F32R, tag="wt")
        nc.sync.dma_start(out=wq_sb, in_=wqt[:, :].rearrange("(c p) m -> p c m", p=128))
        dect_sb = []
        for ci in range(KC):
            t = xt_pool.tile([128, LQ], F32R, tag="xT", name="xt")
            nc.sync.dma_start(out=t, in_=dect[ci * 128:(ci + 1) * 128, :])
            dect_sb.append(t)
        for j in range(2):
            for qb in range(4):
                acc = ps.tile([128, 512], F32, tag="ps", name="acc")
                for ci in range(KC):
                    nc.tensor.matmul(
                        acc,
                        lhsT=wq_sb[:, ci, j * 128:(j + 1) * 128],
                        rhs=dect_sb[ci][:, qb * 512:(qb + 1) * 512],
                        start=(ci == 0),
                        stop=(ci == KC - 1),
                    )
                nc.scalar.copy(out=QT[j][:, qb * 512:(qb + 1) * 512], in_=acc)

        # ---- load encT, K projection ----
        wk_sb = wt_pool.tile([128, KC, 256], F32R, tag="wt")
        nc.sync.dma_start(out=wk_sb, in_=wkt[:, :].rearrange("(c p) m -> p c m", p=128))
        enct_sb = []
        for ci in range(KC):
            t = xt_pool.tile([128, LK], F32R, tag="xT", name="xt")
            nc.sync.dma_start(out=t, in_=enct[ci * 128:(ci + 1) * 128, :])
            enct_sb.append(t)
        # zero the pad halves once (scale=0 copy; reads mb_sb just as a source)
        zsrc = mb_sb[:, 0:1].broadcast_to([128, 2048])
        for j in range(2):
            nc.scalar.activation(
                KT[2 * j][64:128, :], zsrc[64:128, :],
                mybir.ActivationFunctionType.Copy, scale=0.0,
            )
            nc.scalar.activation(
                KT[2 * j + 1][0:64, :], zsrc[0:64, :],
                mybir.ActivationFunctionType.Copy, scale=0.0,
            )
        for j in range(2):
            for kb in range(4):
                acc = ps.tile([128, 512], F32, tag="ps", name="acc")
                for ci in range(KC):
                    nc.tensor.matmul(
                        acc,
                        lhsT=wk_sb[:, ci, j * 128:(j + 1) * 128],
                        rhs=enct_sb[ci][:, kb * 512:(kb + 1) * 512],
                        start=(ci == 0),
                        stop=(ci == KC - 1),
                    )
                nc.scalar.copy(
                    out=KT[2 * j][0:64, kb * 512:(kb + 1) * 512], in_=acc[0:64, :]
                )
                nc.scalar.copy(
                    out=KT[2 * j + 1][64:128, kb * 512:(kb + 1) * 512],
                    in_=acc[64:128, :],
                )

        # ---- V projection (native [k, hd] layout + ones column) ----
        wv_sb = wt_pool.tile([128, KC, 256], F32R, tag="wt")
        nc.sync.dma_start(out=wv_sb, in_=wvt[:, :].rearrange("(c p) m -> p c m", p=128))
        for kt_i in range(NKT):
            acc = ps.tile([128, 256], F32, tag="ps", name="accv")
            for ci in range(KC):
                nc.tensor.matmul(
                    acc,
                    lhsT=enct_sb[ci][:, kt_i * 128:(kt_i + 1) * 128],
                    rhs=wv_sb[:, ci, :],
                    start=(ci == 0),
                    stop=(ci == KC - 1),
                )
            nc.vector.tensor_copy(
                out=v_all[:, kt_i, :, 0:HD],
                in_=acc.rearrange("p (h d) -> p h d", h=HPC),
            )

        # ---- attention: per q-half, per head pair; PV one chunk behind ----
        for qh in range(2 if _PHASES != "proj" else 0):
            q0 = qh * 1024
            for j in range(2):
                cps = [
                    pc.tile([HD + 1, 1024], F32, tag="pc", name=f"cps{s_}")
                    for s_ in range(2)
                ]
                prev = None  # es of previous chunk
                for ci in range(NKT + 1):
                    if ci < NKT:
                        k0 = ci * 128
                        ss = [
                            ps.tile([128, 1024], F32, tag="ps", name="s")
                            for s_ in range(2)
                        ]
                        for qb in range(2):
                            for sub in range(2):
                                nc.tensor.matmul(
                                    ss[sub][:, qb * 512:(qb + 1) * 512],
                                    lhsT=KT[2 * j + sub][:, k0:k0 + 128],
                                    rhs=QT[j][:, q0 + qb * 512:q0 + (qb + 1) * 512],
                                    start=True,
                                    stop=True,
                                )
                        es = []
                        for sub in range(2):
                            e = epool.tile([128, 1024], F16, tag="e", name="e")
                            nc.scalar.activation(
                                e, ss[sub], EXP, bias=mb_sb[:, ci:ci + 1], scale=0.125
                            )
                            es.append(e)
                    # PV + weights DMA for the previous chunk
                    if prev is not None:
                        pes = prev
                        pci = ci - 1
                        pk0 = pci * 128
                        for sub in range(2):
                            h = 2 * j + sub
                            for qb in range(2):
                                nc.tensor.matmul(
                                    cps[sub][:, qb * 512:(qb + 1) * 512],
                                    lhsT=v_all[:, pci, h, :],
                                    rhs=pes[sub][:, qb * 512:(qb + 1) * 512],
                                    start=(pci == 0),
                                    stop=(pci == NKT - 1),
                                )
                            nc.sync.dma_start(
                                out=et[h, pk0:pk0 + 128, q0:q0 + 1024], in_=pes[sub]
                            )
                    prev = es if ci < NKT else None
                # normalize context rows: CT_h = C_unnorm / denom
                for sub in range(2 if _PHASES not in ("attn",) else 0):
                    h = 2 * j + sub
                    nc.scalar.copy(out=dn[0:1, :], in_=cps[sub][HD:HD + 1, :])
                    rb = ps.tile([HD, 1024], F32, tag="ps", name="rb")
                    for qb in range(2):
                        nc.tensor.matmul(
                            rb[:, qb * 512:(qb + 1) * 512],
                            lhsT=ones_sb,
                            rhs=dn[:, qb * 512:(qb + 1) * 512],
                            start=True,
                            stop=True,
                        )
                    rec = rpool.tile([HD, 1024], F32, tag="rec", name="rec")
                    nc.vector.reciprocal_approx_fast(out=rec, in_=rb)
                    nc.vector.tensor_mul(
                        CT[h][:, q0:q0 + 1024], cps[sub][0:HD, :], rec
                    )

        # ---- output projection: outp[q, o] = sum_h CT_h.T @ Wo_h ----
        for qt in range(16 if _PHASES in ("full", "noct") else 0):
            for ob in range(2):
                acc = ps.tile([128, 512], F32, tag="ps", name="acc")
                for h in range(HPC):
                    nc.tensor.matmul(
                        acc,
                        lhsT=CT[h][:, qt * 128:(qt + 1) * 128],
                        rhs=wo_sb[:, h, ob * 512:(ob + 1) * 512],
                        start=(h == 0),
                        stop=(h == HPC - 1),
                    )
                o = opool.tile([128, 512], F32, tag="osb", name="o")
                nc.vector.tensor_copy(out=o, in_=acc)
                nc.sync.dma_start(
                    out=outp[qt * 128:(qt + 1) * 128, ob * 512:(ob + 1) * 512], in_=o
                )


def _build():
    nc = bacc.Bacc(
        "TRN2",
        target_bir_lowering=False,
        debug=False,
        enable_asserts=True,
        num_devices=NCORES,
    )
    dect = nc.declare_dram_parameter("dect", [H, LQ], F32R, isOutput=False)
    enct = nc.declare_dram_parameter("enct", [H, LK], F32R, isOutput=False)
    wqt = nc.declare_dram_parameter("wqt", [H, 256], F32R, isOutput=False)
    wkt = nc.declare_dram_parameter("wkt", [H, 256], F32R, isOutput=False)
    wvt = nc.declare_dram_parameter("wvt", [H, 256], F32R, isOutput=False)
    wot = nc.declare_dram_parameter("wot", [HD, HPC, H], F16, isOutput=False)
    mb = nc.declare_dram_parameter("maskb", [128, NKT], F32, isOutput=False)
    ones_d = nc.declare_dram_parameter("ones", [128, HD], F32R, isOutput=False)
    dnz_d = nc.declare_dram_parameter("dnz", [128, 1024], F32R, isOutput=False)
    et = nc.declare_dram_parameter("et", [HPC, LK, LQ], F16, isOutput=True)
    outp = nc.declare_dram_parameter("outp", [LQ, H], F32, isOutput=True)

    with tile.TileContext(nc) as tc:
        _emit(nc, dect, enct, wqt, wkt, wvt, wot, mb, ones_d, dnz_d, et, outp, tc)
    nc.compile()
    return nc


def _get_nc():
    global _NC
    if _NC is None:
        _NC = _build()
    return _NC


def _in_maps(dec, enc, mask, Wq, Wk, Wv, Wo):
    decT = [np.ascontiguousarray(dec[b].T) for b in range(B)]
    encT = [np.ascontiguousarray(enc[b].T) for b in range(B)]
    maskb = []
    for b in range(B):
        bias = np.where(mask[b] != 0, 0.0, -1e30).astype(np.float32)
        maskb.append(np.ascontiguousarray(bias.reshape(NKT, 128).T))
    maps = []
    for c in range(NCORES):
        b, g = c // 4, c % 4
        sl = slice(g * 256, (g + 1) * 256)
        wot_c = np.ascontiguousarray(
            Wo[:, sl].T.reshape(HPC, HD, H).transpose(1, 0, 2).astype(np.float16)
        )
        maps.append(
            {
                "dect": decT[b],
                "enct": encT[b],
                "wqt": np.ascontiguousarray(Wq[sl, :].T),
                "wkt": np.ascontiguousarray(Wk[sl, :].T),
                "wvt": np.ascontiguousarray(Wv[sl, :].T),
                "wot": wot_c,
                "maskb": maskb[b],
                "ones": _ONES_Z,
                "dnz": _DN_Z,
            }
        )
    return maps


def _assemble(results, bo):
    out = np.empty((B, LQ, H), np.float32)
    weights = np.empty((B, NH, LQ, LK), np.float32)
    for b in range(B):
        acc = results[b * 4]["outp"].astype(np.float32, copy=True)
        for g in range(1, 4):
            acc += results[b * 4 + g]["outp"]
        out[b] = acc + bo[None, :]
    for c in range(NCORES):
        b, g = c // 4, c % 4
        E16 = results[c]["et"]  # [HPC, k, q] f16
        for hl in range(HPC):
            Ef = E16[hl].astype(np.float32)
            denom = Ef.sum(axis=0)
            weights[b, g * 4 + hl] = (Ef / denom[None, :]).T
    return out, weights


def _run(inputs, trace=False):
    dec = np.asarray(inputs["decoder_hidden"], np.float32)
    enc = np.asarray(inputs["encoder_hidden"], np.float32)
    mask = np.asarray(inputs["encoder_attention_mask"])
    Wq = np.asarray(inputs["Wq"], np.float32)
    Wk = np.asarray(inputs["Wk"], np.float32)
    Wv = np.asarray(inputs["Wv"], np.float32)
    Wo = np.asarray(inputs["Wo"], np.float32)
    bo = np.asarray(inputs["bo"], np.float32)

    nc = _get_nc()
    maps = _in_maps(dec, enc, mask, Wq, Wk, Wv, Wo)
    res = run_bass_kernel_spmd(nc, maps, list(range(NCORES)), trace=trace)
    out, weights = _assemble(res.results, bo)
    return out, weights, res


def kernel(
    decoder_hidden,
    encoder_hidden,
    encoder_attention_mask,
    Wq,
    Wk,
    Wv,
    Wo,
    bo,
):
    out, weights, _ = _run(
        {
            "decoder_hidden": decoder_hidden,
            "encoder_hidden": encoder_hidden,
            "encoder_attention_mask": encoder_attention_mask,
            "Wq": Wq,
            "Wk": Wk,
            "Wv": Wv,
            "Wo": Wo,
            "bo": bo,
        }
    )
    return out, weights


# revision 10
# speedup vs baseline: 1.7656x; 1.1480x over previous
"""Cross-attention layer (B=2, Lq=Lk=2048, H=1024, NH=16) on 8 TRN2 NeuronCores.

Sharding: core c handles batch b = c//4 and the 4 heads g*4..g*4+3 where
g = c%4 (data parallel on B x tensor parallel on heads).

Per-core device program (SPMD, identical program, different data):
  - Q/K projections computed in transposed layout QT/KT [head_dim, L]
    directly from host-pretransposed activations decT/encT [H, L]
    (contraction over H lives on the partition axis; fp32r matmuls).
  - V projection in native [k, head_dim] layout, stored f16 with an
    appended ones-column (augmented-V trick: P@V_aug also yields the
    softmax denominator as row 64 of the PSUM accumulator).
  - Scores are computed transposed, S^T = K @ Q^T [k, q], so the softmax
    exp (with mask bias per k-partition and 1/8 scale fused into the ACT
    instruction) needs no reductions at all. The two heads of a pair use
    disjoint PE row groups (K=64) and are emitted adjacently so they can
    run concurrently on the PE's 32x32 subarrays.
  - The P@V matmul runs one k-chunk behind the scores (software
    pipeline) so the scheduler never splits a score pair.
  - E = exp(S^T) f16 goes three ways: DMA to DRAM (the attention-weights
    output, normalized + transposed on host), rhs of the P@V_aug matmul,
    and its column sums (via the ones row) become the denominators.
  - Context rows are normalized with a reciprocal + PE-broadcast and fed
    into the output projection; per-core partial outputs are summed on
    host (tensor-parallel reduce) together with the bias.
"""

import os
import sys

sys.path.insert(0, "/opt/trn_rl_repo")

import numpy as np

import concourse.bacc as bacc
import concourse.mybir as mybir
import concourse.tile as tile
from concourse.bass_utils import run_bass_kernel_spmd

_PHASES = os.environ.get("KBUILD_PHASES", "full")

B, LQ, LK, H, NH = 2, 2048, 2048, 1024, 16
HD = H // NH          # 64
NCORES = 8
HPC = NH // 4         # 4 heads per core
KC = H // 128         # 8 hidden-dim chunks
NKT = LK // 128       # 16 key tiles

F32 = mybir.dt.float32
F32R = mybir.dt.float32r
F16 = mybir.dt.float16
EXP = mybir.ActivationFunctionType.Exp

_NC = None
_ONES_Z = np.zeros((128, HD), np.float32)
_ONES_Z[0, :] = 1.0
_DN_Z = np.zeros((128, 1024), np.float32)


def _emit(nc, dect, enct, wqt, wkt, wvt, wot, mb, ones_d, dnz_d, et, outp, tc):
    with (
        tc.tile_pool(name="xTd", bufs=KC) as xtd_pool,
        tc.tile_pool(name="xTe", bufs=KC) as xte_pool,
        tc.tile_pool(name="wt", bufs=3) as wt_pool,
        tc.tile_pool(name="pp", bufs=1) as pp,
        tc.tile_pool(name="e", bufs=5) as epool,
        tc.tile_pool(name="r", bufs=1) as rpool,
        tc.tile_pool(name="osb", bufs=2) as opool,
        tc.tile_pool(name="ps", bufs=2, space="PSUM") as ps,
        tc.tile_pool(name="pc", bufs=2, space="PSUM") as pc,
    ):
        # ---- persistent tiles ----
        mb_sb = pp.tile([128, NKT], F32, tag="mb")
        nc.sync.dma_start(out=mb_sb, in_=mb[:, :])
        ones_sb = pp.tile([128, HD], F32R, tag="ones")
        nc.sync.dma_start(out=ones_sb, in_=ones_d[:, :])
        dn = pp.tile([128, 1024], F32R, tag="dn")
        nc.sync.dma_start(out=dn, in_=dnz_d[:, :])
        wo_sb = pp.tile([128, HPC, H], F16, tag="wo")
        nc.sync.dma_start(out=wo_sb, in_=wot[:, :, :])

        QT = [pp.tile([128, LQ], F32R, tag=f"qt{j}", name=f"qt{j}") for j in range(2)]
        # per-head K tiles, zero-padded on the other head's partitions so the
        # score matmuls run with K=128 (no PE tiling-mode switches)
        KT = [pp.tile([128, LK], F32R, tag=f"kt{h}", name=f"kt{h}") for h in range(HPC)]
        CT = [pp.tile([128, LQ], F16, tag=f"ct{h}", name=f"ct{h}") for h in range(HPC)]
        for h in range(HPC):
            nc.vector.memset(CT[h][HD:128, :], 0.0)
        v_all = pp.tile([128, NKT, HPC, HD + 1], F16, tag="vall")
        nc.vector.memset(v_all[:, :, :, HD], 1.0)

        # ---- load all inputs up front (f16 halves the DMA bytes) ----
        wq_sb = wt_pool.tile([128, KC, 256], F16, tag="wt")
        nc.sync.dma_start(out=wq_sb, in_=wqt[:, :].rearrange("(c p) m -> p c m", p=128))
        wk_sb = wt_pool.tile([128, KC, 256], F16, tag="wt")
        nc.sync.dma_start(out=wk_sb, in_=wkt[:, :].rearrange("(c p) m -> p c m", p=128))
        wv_sb = wt_pool.tile([128, KC, 256], F16, tag="wt")
        nc.sync.dma_start(out=wv_sb, in_=wvt[:, :].rearrange("(c p) m -> p c m", p=128))
        dect_sb = []
        for ci in range(KC):
            t = xtd_pool.tile([128, LQ], F16, tag="xTd", name="xtd")
            nc.sync.dma_start(out=t, in_=dect[ci * 128:(ci + 1) * 128, :])
            dect_sb.append(t)
        enct_sb = []
        for ci in range(KC):
            t = xte_pool.tile([128, LK], F16, tag="xTe", name="xte")
            nc.sync.dma_start(out=t, in_=enct[ci * 128:(ci + 1) * 128, :])
            enct_sb.append(t)
        for j in range(2):
            for qb in range(4):
                acc = ps.tile([128, 512], F32, tag="ps", name="acc")
                for ci in range(KC):
                    nc.tensor.matmul(
                        acc,
                        lhsT=wq_sb[:, ci, j * 128:(j + 1) * 128],
                        rhs=dect_sb[ci][:, qb * 512:(qb + 1) * 512],
                        start=(ci == 0),
                        stop=(ci == KC - 1),
                    )
                nc.scalar.copy(out=QT[j][:, qb * 512:(qb + 1) * 512], in_=acc)

        # ---- K projection ----
        # zero the pad halves once (scale=0 copy; reads mb_sb just as a source)
        zsrc = mb_sb[:, 0:1].broadcast_to([128, 2048])
        for j in range(2):
            nc.scalar.activation(
                KT[2 * j][64:128, :], zsrc[64:128, :],
                mybir.ActivationFunctionType.Copy, scale=0.0,
            )
            nc.scalar.activation(
                KT[2 * j + 1][0:64, :], zsrc[0:64, :],
                mybir.ActivationFunctionType.Copy, scale=0.0,
            )
        for j in range(2):
            for kb in range(4):
                acc = ps.tile([128, 512], F32, tag="ps", name="acc")
                for ci in range(KC):
                    nc.tensor.matmul(
                        acc,
                        lhsT=wk_sb[:, ci, j * 128:(j + 1) * 128],
                        rhs=enct_sb[ci][:, kb * 512:(kb + 1) * 512],
                        start=(ci == 0),
                        stop=(ci == KC - 1),
                    )
                nc.scalar.copy(
                    out=KT[2 * j][0:64, kb * 512:(kb + 1) * 512], in_=acc[0:64, :]
                )
                nc.scalar.copy(
                    out=KT[2 * j + 1][64:128, kb * 512:(kb + 1) * 512],
                    in_=acc[64:128, :],
                )

        # ---- V projection (native [k, hd] layout + ones column) ----
        for kt_i in range(NKT):
            acc = ps.tile([128, 256], F32, tag="ps", name="accv")
            for ci in range(KC):
                nc.tensor.matmul(
                    acc,
                    lhsT=enct_sb[ci][:, kt_i * 128:(kt_i + 1) * 128],
                    rhs=wv_sb[:, ci, :],
                    start=(ci == 0),
                    stop=(ci == KC - 1),
                )
            nc.vector.tensor_copy(
                out=v_all[:, kt_i, :, 0:HD],
                in_=acc.rearrange("p (h d) -> p h d", h=HPC),
            )

        # ---- attention: per q-half, per head pair; PV one chunk behind ----
        for qh in range(2 if _PHASES != "proj" else 0):
            q0 = qh * 1024
            for j in range(2):
                cps = [
                    pc.tile([HD + 1, 1024], F32, tag="pc", name=f"cps{s_}")
                    for s_ in range(2)
                ]
                prev = None  # es of previous chunk
                for ci in range(NKT + 1):
                    if ci < NKT:
                        k0 = ci * 128
                        ss = [
                            ps.tile([128, 1024], F32, tag="ps", name="s")
                            for s_ in range(2)
                        ]
                        for qb in range(2):
                            for sub in range(2):
                                nc.tensor.matmul(
                                    ss[sub][:, qb * 512:(qb + 1) * 512],
                                    lhsT=KT[2 * j + sub][:, k0:k0 + 128],
                                    rhs=QT[j][:, q0 + qb * 512:q0 + (qb + 1) * 512],
                                    start=True,
                                    stop=True,
                                )
                        es = []
                        for sub in range(2):
                            e = epool.tile([128, 1024], F16, tag="e", name="e")
                            nc.scalar.activation(
                                e, ss[sub], EXP, bias=mb_sb[:, ci:ci + 1], scale=0.125
                            )
                            es.append(e)
                    # PV + weights DMA for the previous chunk
                    if prev is not None:
                        pes = prev
                        pci = ci - 1
                        pk0 = pci * 128
                        for sub in range(2):
                            h = 2 * j + sub
                            for qb in range(2):
                                nc.tensor.matmul(
                                    cps[sub][:, qb * 512:(qb + 1) * 512],
                                    lhsT=v_all[:, pci, h, :],
                                    rhs=pes[sub][:, qb * 512:(qb + 1) * 512],
                                    start=(pci == 0),
                                    stop=(pci == NKT - 1),
                                )
                            nc.sync.dma_start(
                                out=et[h, pk0:pk0 + 128, q0:q0 + 1024], in_=pes[sub]
                            )
                    prev = es if ci < NKT else None
                # normalize context rows: CT_h = C_unnorm / denom
                for sub in range(2 if _PHASES not in ("attn",) else 0):
                    h = 2 * j + sub
                    nc.scalar.copy(out=dn[0:1, :], in_=cps[sub][HD:HD + 1, :])
                    rb = ps.tile([HD, 1024], F32, tag="ps", name="rb")
                    for qb in range(2):
                        nc.tensor.matmul(
                            rb[:, qb * 512:(qb + 1) * 512],
                            lhsT=ones_sb,
                            rhs=dn[:, qb * 512:(qb + 1) * 512],
                            start=True,
                            stop=True,
                        )
                    rec = rpool.tile([HD, 1024], F32, tag="rec", name="rec")
                    nc.vector.reciprocal_approx_fast(out=rec, in_=rb)
                    nc.vector.tensor_mul(
                        CT[h][0:HD, q0:q0 + 1024], cps[sub][0:HD, :], rec
                    )

        # ---- output projection: outp[q, o] = sum_h CT_h.T @ Wo_h ----
        for qt in range(16 if _PHASES in ("full", "noct") else 0):
            for ob in range(2):
                acc = ps.tile([128, 512], F32, tag="ps", name="acc")
                for h in range(HPC):
                    nc.tensor.matmul(
                        acc,
                        lhsT=CT[h][:, qt * 128:(qt + 1) * 128],
                        rhs=wo_sb[:, h, ob * 512:(ob + 1) * 512],
                        start=(h == 0),
                        stop=(h == HPC - 1),
                    )
                o = opool.tile([128, 512], F32, tag="osb", name="o")
                nc.vector.tensor_copy(out=o, in_=acc)
                nc.sync.dma_start(
                    out=outp[qt * 128:(qt + 1) * 128, ob * 512:(ob + 1) * 512], in_=o
                )


def _build():
    nc = bacc.Bacc(
        "TRN2",
        target_bir_lowering=False,
        debug=False,
        enable_asserts=True,
        num_devices=NCORES,
    )
    dect = nc.declare_dram_parameter("dect", [H, LQ], F16, isOutput=False)
    enct = nc.declare_dram_parameter("enct", [H, LK], F16, isOutput=False)
    wqt = nc.declare_dram_parameter("wqt", [H, 256], F16, isOutput=False)
    wkt = nc.declare_dram_parameter("wkt", [H, 256], F16, isOutput=False)
    wvt = nc.declare_dram_parameter("wvt", [H, 256], F16, isOutput=False)
    wot = nc.declare_dram_parameter("wot", [128, HPC, H], F16, isOutput=False)
    mb = nc.declare_dram_parameter("maskb", [128, NKT], F32, isOutput=False)
    ones_d = nc.declare_dram_parameter("ones", [128, HD], F32R, isOutput=False)
    dnz_d = nc.declare_dram_parameter("dnz", [128, 1024], F32R, isOutput=False)
    et = nc.declare_dram_parameter("et", [HPC, LK, LQ], F16, isOutput=True)
    outp = nc.declare_dram_parameter("outp", [LQ, H], F32, isOutput=True)

    with tile.TileContext(nc) as tc:
        _emit(nc, dect, enct, wqt, wkt, wvt, wot, mb, ones_d, dnz_d, et, outp, tc)
    nc.compile()
    return nc


def _get_nc():
    global _NC
    if _NC is None:
        _NC = _build()
    return _NC


def _in_maps(dec, enc, mask, Wq, Wk, Wv, Wo):
    decT = [np.ascontiguousarray(dec[b].T.astype(np.float16)) for b in range(B)]
    encT = [np.ascontiguousarray(enc[b].T.astype(np.float16)) for b in range(B)]
    maskb = []
    for b in range(B):
        bias = np.where(mask[b] != 0, 0.0, -1e30).astype(np.float32)
        maskb.append(np.ascontiguousarray(bias.reshape(NKT, 128).T))
    maps = []
    for c in range(NCORES):
        b, g = c // 4, c % 4
        sl = slice(g * 256, (g + 1) * 256)
        wot_c = np.zeros((128, HPC, H), np.float16)
        wot_c[0:HD] = Wo[:, sl].T.reshape(HPC, HD, H).transpose(1, 0, 2)
        maps.append(
            {
                "dect": decT[b],
                "enct": encT[b],
                "wqt": np.ascontiguousarray(Wq[sl, :].T.astype(np.float16)),
                "wkt": np.ascontiguousarray(Wk[sl, :].T.astype(np.float16)),
                "wvt": np.ascontiguousarray(Wv[sl, :].T.astype(np.float16)),
                "wot": wot_c,
                "maskb": maskb[b],
                "ones": _ONES_Z,
                "dnz": _DN_Z,
            }
        )
    return maps


def _assemble(results, bo):
    out = np.empty((B, LQ, H), np.float32)
    weights = np.empty((B, NH, LQ, LK), np.float32)
    for b in range(B):
        acc = results[b * 4]["outp"].astype(np.float32, copy=True)
        for g in range(1, 4):
            acc += results[b * 4 + g]["outp"]
        out[b] = acc + bo[None, :]
    for c in range(NCORES):
        b, g = c // 4, c % 4
        E16 = results[c]["et"]  # [HPC, k, q] f16
        for hl in range(HPC):
            Ef = E16[hl].astype(np.float32)
            denom = Ef.sum(axis=0)
            weights[b, g * 4 + hl] = (Ef / denom[None, :]).T
    return out, weights


def _run(inputs, trace=False):
    dec = np.asarray(inputs["decoder_hidden"], np.float32)
    enc = np.asarray(inputs["encoder_hidden"], np.float32)
    mask = np.asarray(inputs["encoder_attention_mask"])
    Wq = np.asarray(inputs["Wq"], np.float32)
    Wk = np.asarray(inputs["Wk"], np.float32)
    Wv = np.asarray(inputs["Wv"], np.float32)
    Wo = np.asarray(inputs["Wo"], np.float32)
    bo = np.asarray(inputs["bo"], np.float32)

    nc = _get_nc()
    maps = _in_maps(dec, enc, mask, Wq, Wk, Wv, Wo)
    res = run_bass_kernel_spmd(nc, maps, list(range(NCORES)), trace=trace)
    out, weights = _assemble(res.results, bo)
    return out, weights, res


def kernel(
    decoder_hidden,
    encoder_hidden,
    encoder_attention_mask,
    Wq,
    Wk,
    Wv,
    Wo,
    bo,
):
    out, weights, _ = _run(
        {
            "decoder_hidden": decoder_hidden,
            "encoder_hidden": encoder_hidden,
            "encoder_attention_mask": encoder_attention_mask,
            "Wq": Wq,
            "Wk": Wk,
            "Wv": Wv,
            "Wo": Wo,
            "bo": bo,
        }
    )
    return out, weights


# revision 11
# speedup vs baseline: 1.7710x; 1.0031x over previous
"""Cross-attention layer (B=2, Lq=Lk=2048, H=1024, NH=16) on 8 TRN2 NeuronCores.

Sharding: core c handles batch b = c//4 and the 4 heads g*4..g*4+3 where
g = c%4 (data parallel on B x tensor parallel on heads).

Per-core device program (SPMD, identical program, different data):
  - Q/K projections computed in transposed layout QT/KT [head_dim, L]
    directly from host-pretransposed activations decT/encT [H, L]
    (contraction over H lives on the partition axis; fp32r matmuls).
  - V projection in native [k, head_dim] layout, stored f16 with an
    appended ones-column (augmented-V trick: P@V_aug also yields the
    softmax denominator as row 64 of the PSUM accumulator).
  - Scores are computed transposed, S^T = K @ Q^T [k, q], so the softmax
    exp (with mask bias per k-partition and 1/8 scale fused into the ACT
    instruction) needs no reductions at all. The two heads of a pair use
    disjoint PE row groups (K=64) and are emitted adjacently so they can
    run concurrently on the PE's 32x32 subarrays.
  - The P@V matmul runs one k-chunk behind the scores (software
    pipeline) so the scheduler never splits a score pair.
  - E = exp(S^T) f16 goes three ways: DMA to DRAM (the attention-weights
    output, normalized + transposed on host), rhs of the P@V_aug matmul,
    and its column sums (via the ones row) become the denominators.
  - Context rows are normalized with a reciprocal + PE-broadcast and fed
    into the output projection; per-core partial outputs are summed on
    host (tensor-parallel reduce) together with the bias.
"""

import os
import sys

sys.path.insert(0, "/opt/trn_rl_repo")

import numpy as np

import concourse.bacc as bacc
import concourse.mybir as mybir
import concourse.tile as tile
from concourse.bass_utils import run_bass_kernel_spmd

_PHASES = os.environ.get("KBUILD_PHASES", "full")

B, LQ, LK, H, NH = 2, 2048, 2048, 1024, 16
HD = H // NH          # 64
NCORES = 8
HPC = NH // 4         # 4 heads per core
KC = H // 128         # 8 hidden-dim chunks
NKT = LK // 128       # 16 key tiles

F32 = mybir.dt.float32
F32R = mybir.dt.float32r
F16 = mybir.dt.float16
EXP = mybir.ActivationFunctionType.Exp

_NC = None
_ONES_Z = np.zeros((128, HD), np.float32)
_ONES_Z[0, :] = 1.0
_DN_Z = np.zeros((128, 1024), np.float32)


def _emit(nc, dect, enct, wqt, wkt, wvt, wot, mb, ones_d, dnz_d, et, outp, tc):
    with (
        tc.tile_pool(name="xTd", bufs=KC) as xtd_pool,
        tc.tile_pool(name="xTe", bufs=KC) as xte_pool,
        tc.tile_pool(name="wt", bufs=3) as wt_pool,
        tc.tile_pool(name="pp", bufs=1) as pp,
        tc.tile_pool(name="e", bufs=5) as epool,
        tc.tile_pool(name="r", bufs=1) as rpool,
        tc.tile_pool(name="osb", bufs=2) as opool,
        tc.tile_pool(name="ps", bufs=2, space="PSUM") as ps,
        tc.tile_pool(name="pc", bufs=2, space="PSUM") as pc,
    ):
        # ---- persistent tiles ----
        mb_sb = pp.tile([128, NKT], F32, tag="mb")
        nc.sync.dma_start(out=mb_sb, in_=mb[:, :])
        ones_sb = pp.tile([128, HD], F32R, tag="ones")
        nc.sync.dma_start(out=ones_sb, in_=ones_d[:, :])
        dn = pp.tile([128, 1024], F32R, tag="dn")
        nc.sync.dma_start(out=dn, in_=dnz_d[:, :])
        wo_sb = pp.tile([128, 2, H], F16, tag="wo")
        nc.sync.dma_start(out=wo_sb, in_=wot[:, :, :])

        QT = [pp.tile([128, LQ], F32R, tag=f"qt{j}", name=f"qt{j}") for j in range(2)]
        # per-head K tiles, zero-padded on the other head's partitions so the
        # score matmuls run with K=128 (no PE tiling-mode switches)
        KT = [pp.tile([128, LK], F32R, tag=f"kt{h}", name=f"kt{h}") for h in range(HPC)]
        CT = [
            [pp.tile([128, 1024], F16, tag=f"ct{j}{qh}", name=f"ct{j}{qh}") for qh in range(2)]
            for j in range(2)
        ]
        v_all = pp.tile([128, NKT, HPC, HD + 1], F16, tag="vall")
        nc.vector.memset(v_all[:, :, :, HD], 1.0)

        # ---- load all inputs up front (f16 halves the DMA bytes) ----
        wq_sb = wt_pool.tile([128, KC, 256], F16, tag="wt")
        nc.sync.dma_start(out=wq_sb, in_=wqt[:, :].rearrange("(c p) m -> p c m", p=128))
        wk_sb = wt_pool.tile([128, KC, 256], F16, tag="wt")
        nc.sync.dma_start(out=wk_sb, in_=wkt[:, :].rearrange("(c p) m -> p c m", p=128))
        wv_sb = wt_pool.tile([128, KC, 256], F16, tag="wt")
        nc.sync.dma_start(out=wv_sb, in_=wvt[:, :].rearrange("(c p) m -> p c m", p=128))
        dect_sb = []
        for ci in range(KC):
            t = xtd_pool.tile([128, LQ], F16, tag="xTd", name="xtd")
            nc.sync.dma_start(out=t, in_=dect[ci * 128:(ci + 1) * 128, :])
            dect_sb.append(t)
        enct_sb = []
        for ci in range(KC):
            t = xte_pool.tile([128, LK], F16, tag="xTe", name="xte")
            nc.sync.dma_start(out=t, in_=enct[ci * 128:(ci + 1) * 128, :])
            enct_sb.append(t)
        for j in range(2):
            for qb in range(4):
                acc = ps.tile([128, 512], F32, tag="ps", name="acc")
                for ci in range(KC):
                    nc.tensor.matmul(
                        acc,
                        lhsT=wq_sb[:, ci, j * 128:(j + 1) * 128],
                        rhs=dect_sb[ci][:, qb * 512:(qb + 1) * 512],
                        start=(ci == 0),
                        stop=(ci == KC - 1),
                    )
                nc.scalar.copy(out=QT[j][:, qb * 512:(qb + 1) * 512], in_=acc)

        # ---- K projection ----
        # zero the pad halves once (scale=0 copy; reads mb_sb just as a source)
        zsrc = mb_sb[:, 0:1].broadcast_to([128, 2048])
        for j in range(2):
            nc.scalar.activation(
                KT[2 * j][64:128, :], zsrc[64:128, :],
                mybir.ActivationFunctionType.Copy, scale=0.0,
            )
            nc.scalar.activation(
                KT[2 * j + 1][0:64, :], zsrc[0:64, :],
                mybir.ActivationFunctionType.Copy, scale=0.0,
            )
        for j in range(2):
            for kb in range(4):
                acc = ps.tile([128, 512], F32, tag="ps", name="acc")
                for ci in range(KC):
                    nc.tensor.matmul(
                        acc,
                        lhsT=wk_sb[:, ci, j * 128:(j + 1) * 128],
                        rhs=enct_sb[ci][:, kb * 512:(kb + 1) * 512],
                        start=(ci == 0),
                        stop=(ci == KC - 1),
                    )
                nc.scalar.copy(
                    out=KT[2 * j][0:64, kb * 512:(kb + 1) * 512], in_=acc[0:64, :]
                )
                nc.scalar.copy(
                    out=KT[2 * j + 1][64:128, kb * 512:(kb + 1) * 512],
                    in_=acc[64:128, :],
                )

        # ---- V projection (native [k, hd] layout + ones column) ----
        for kt_i in range(NKT):
            acc = ps.tile([128, 256], F32, tag="ps", name="accv")
            for ci in range(KC):
                nc.tensor.matmul(
                    acc,
                    lhsT=enct_sb[ci][:, kt_i * 128:(kt_i + 1) * 128],
                    rhs=wv_sb[:, ci, :],
                    start=(ci == 0),
                    stop=(ci == KC - 1),
                )
            nc.vector.tensor_copy(
                out=v_all[:, kt_i, :, 0:HD],
                in_=acc.rearrange("p (h d) -> p h d", h=HPC),
            )

        # ---- attention: per q-half, per head pair; PV one chunk behind ----
        for qh in range(2 if _PHASES != "proj" else 0):
            q0 = qh * 1024
            for j in range(2):
                cps = [
                    pc.tile([HD + 1, 1024], F32, tag="pc", name=f"cps{s_}")
                    for s_ in range(2)
                ]
                prev = None  # es of previous chunk
                for ci in range(NKT + 1):
                    if ci < NKT:
                        k0 = ci * 128
                        ss = [
                            ps.tile([128, 1024], F32, tag="ps", name="s")
                            for s_ in range(2)
                        ]
                        for qb in range(2):
                            for sub in range(2):
                                nc.tensor.matmul(
                                    ss[sub][:, qb * 512:(qb + 1) * 512],
                                    lhsT=KT[2 * j + sub][:, k0:k0 + 128],
                                    rhs=QT[j][:, q0 + qb * 512:q0 + (qb + 1) * 512],
                                    start=True,
                                    stop=True,
                                )
                        es = []
                        for sub in range(2):
                            e = epool.tile([128, 1024], F16, tag="e", name="e")
                            nc.scalar.activation(
                                e, ss[sub], EXP, bias=mb_sb[:, ci:ci + 1], scale=0.125
                            )
                            es.append(e)
                    # PV + weights DMA for the previous chunk
                    if prev is not None:
                        pes = prev
                        pci = ci - 1
                        pk0 = pci * 128
                        for sub in range(2):
                            h = 2 * j + sub
                            for qb in range(2):
                                nc.tensor.matmul(
                                    cps[sub][:, qb * 512:(qb + 1) * 512],
                                    lhsT=v_all[:, pci, h, :],
                                    rhs=pes[sub][:, qb * 512:(qb + 1) * 512],
                                    start=(pci == 0),
                                    stop=(pci == NKT - 1),
                                )
                            nc.sync.dma_start(
                                out=et[h, pk0:pk0 + 128, q0:q0 + 1024], in_=pes[sub]
                            )
                    prev = es if ci < NKT else None
                # normalize context rows: CT_h = C_unnorm / denom
                for sub in range(2 if _PHASES not in ("attn",) else 0):
                    nc.scalar.copy(out=dn[0:1, :], in_=cps[sub][HD:HD + 1, :])
                    rb = ps.tile([HD, 1024], F32, tag="ps", name="rb")
                    for qb in range(2):
                        nc.tensor.matmul(
                            rb[:, qb * 512:(qb + 1) * 512],
                            lhsT=ones_sb,
                            rhs=dn[:, qb * 512:(qb + 1) * 512],
                            start=True,
                            stop=True,
                        )
                    rec = rpool.tile([HD, 1024], F32, tag="rec", name="rec")
                    nc.vector.reciprocal_approx_fast(out=rec, in_=rb)
                    if sub == 0:
                        nc.vector.tensor_mul(
                            CT[j][qh][0:HD, :], cps[sub][0:HD, :], rec
                        )
                    else:
                        ctmp = rpool.tile([HD, 1024], F16, tag="ctmp", name="ctmp")
                        nc.vector.tensor_mul(ctmp, cps[sub][0:HD, :], rec)
                        # engines cannot shift partitions; a SBUF->SBUF DMA can
                        nc.sync.dma_start(out=CT[j][qh][HD:128, :], in_=ctmp)

        # ---- output projection: outp[q, o] = sum_j CT2_j.T @ Wo2_j ----
        for qt in range(16 if _PHASES in ("full", "noct") else 0):
            qh = qt // 8
            lq0 = (qt % 8) * 128
            for ob in range(2):
                acc = ps.tile([128, 512], F32, tag="ps", name="acc")
                for j in range(2):
                    nc.tensor.matmul(
                        acc,
                        lhsT=CT[j][qh][:, lq0:lq0 + 128],
                        rhs=wo_sb[:, j, ob * 512:(ob + 1) * 512],
                        start=(j == 0),
                        stop=(j == 1),
                    )
                o = opool.tile([128, 512], F32, tag="osb", name="o")
                nc.vector.tensor_copy(out=o, in_=acc)
                nc.sync.dma_start(
                    out=outp[qt * 128:(qt + 1) * 128, ob * 512:(ob + 1) * 512], in_=o
                )


def _build():
    nc = bacc.Bacc(
        "TRN2",
        target_bir_lowering=False,
        debug=False,
        enable_asserts=True,
        num_devices=NCORES,
    )
    dect = nc.declare_dram_parameter("dect", [H, LQ], F16, isOutput=False)
    enct = nc.declare_dram_parameter("enct", [H, LK], F16, isOutput=False)
    wqt = nc.declare_dram_parameter("wqt", [H, 256], F16, isOutput=False)
    wkt = nc.declare_dram_parameter("wkt", [H, 256], F16, isOutput=False)
    wvt = nc.declare_dram_parameter("wvt", [H, 256], F16, isOutput=False)
    wot = nc.declare_dram_parameter("wot", [128, 2, H], F16, isOutput=False)
    mb = nc.declare_dram_parameter("maskb", [128, NKT], F32, isOutput=False)
    ones_d = nc.declare_dram_parameter("ones", [128, HD], F32R, isOutput=False)
    dnz_d = nc.declare_dram_parameter("dnz", [128, 1024], F32R, isOutput=False)
    et = nc.declare_dram_parameter("et", [HPC, LK, LQ], F16, isOutput=True)
    outp = nc.declare_dram_parameter("outp", [LQ, H], F32, isOutput=True)

    with tile.TileContext(nc) as tc:
        _emit(nc, dect, enct, wqt, wkt, wvt, wot, mb, ones_d, dnz_d, et, outp, tc)
    nc.compile()
    return nc


def _get_nc():
    global _NC
    if _NC is None:
        _NC = _build()
    return _NC


def _in_maps(dec, enc, mask, Wq, Wk, Wv, Wo):
    decT = [np.ascontiguousarray(dec[b].T.astype(np.float16)) for b in range(B)]
    encT = [np.ascontiguousarray(enc[b].T.astype(np.float16)) for b in range(B)]
    maskb = []
    for b in range(B):
        bias = np.where(mask[b] != 0, 0.0, -1e30).astype(np.float32)
        maskb.append(np.ascontiguousarray(bias.reshape(NKT, 128).T))
    maps = []
    for c in range(NCORES):
        b, g = c // 4, c % 4
        sl = slice(g * 256, (g + 1) * 256)
        w4 = Wo[:, sl].T.reshape(2, 2, HD, H)  # [j, sub, d, o]
        wot_c = np.ascontiguousarray(
            w4.transpose(1, 2, 0, 3).reshape(128, 2, H).astype(np.float16)
        )
        maps.append(
            {
                "dect": decT[b],
                "enct": encT[b],
                "wqt": np.ascontiguousarray(Wq[sl, :].T.astype(np.float16)),
                "wkt": np.ascontiguousarray(Wk[sl, :].T.astype(np.float16)),
                "wvt": np.ascontiguousarray(Wv[sl, :].T.astype(np.float16)),
                "wot": wot_c,
                "maskb": maskb[b],
                "ones": _ONES_Z,
                "dnz": _DN_Z,
            }
        )
    return maps


def _assemble(results, bo):
    out = np.empty((B, LQ, H), np.float32)
    weights = np.empty((B, NH, LQ, LK), np.float32)
    for b in range(B):
        acc = results[b * 4]["outp"].astype(np.float32, copy=True)
        for g in range(1, 4):
            acc += results[b * 4 + g]["outp"]
        out[b] = acc + bo[None, :]
    for c in range(NCORES):
        b, g = c // 4, c % 4
        E16 = results[c]["et"]  # [HPC, k, q] f16
        for hl in range(HPC):
            Ef = E16[hl].astype(np.float32)
            denom = Ef.sum(axis=0)
            weights[b, g * 4 + hl] = (Ef / denom[None, :]).T
    return out, weights


def _run(inputs, trace=False):
    dec = np.asarray(inputs["decoder_hidden"], np.float32)
    enc = np.asarray(inputs["encoder_hidden"], np.float32)
    mask = np.asarray(inputs["encoder_attention_mask"])
    Wq = np.asarray(inputs["Wq"], np.float32)
    Wk = np.asarray(inputs["Wk"], np.float32)
    Wv = np.asarray(inputs["Wv"], np.float32)
    Wo = np.asarray(inputs["Wo"], np.float32)
    bo = np.asarray(inputs["bo"], np.float32)

    nc = _get_nc()
    maps = _in_maps(dec, enc, mask, Wq, Wk, Wv, Wo)
    res = run_bass_kernel_spmd(nc, maps, list(range(NCORES)), trace=trace)
    out, weights = _assemble(res.results, bo)
    return out, weights, res


def kernel(
    decoder_hidden,
    encoder_hidden,
    encoder_attention_mask,
    Wq,
    Wk,
    Wv,
    Wo,
    bo,
):
    out, weights, _ = _run(
        {
            "decoder_hidden": decoder_hidden,
            "encoder_hidden": encoder_hidden,
            "encoder_attention_mask": encoder_attention_mask,
            "Wq": Wq,
            "Wk": Wk,
            "Wv": Wv,
            "Wo": Wo,
            "bo": bo,
        }
    )
    return out, weights
